# revision 14
# baseline (speedup 1.0000x reference)
"""MixLinear GEMM kernel for Trainium2 (8 NeuronCores, column-parallel).

Computes, for full inputs:
    inputs = x.reshape(-1, 4096)
    act_outliers = inputs[:, ind]
    inputs_z = inputs with ind-columns zeroed
    x_scale = clamp(rowmax(|inputs_z|)/127, 1e-8)
    q_x = round(inputs_z / x_scale)                  (|q_x| <= 127 by construction)
    y = (q_x @ q_weight.T) * x_scale * scale_col + act_outliers @ weight_cache.T + bias

Device-side formulation: the host pre-packs a combined fp16 weight
    Wc[k, o] = q_weight[o, k] * scale_col[o]          for k not in ind
    Wc[k, o] = sum_{j: ind[j]==k} weight_cache[o, j]  for k in ind
so that with qb[m, k] = round(x[m, k] / xs[m]) + 1536 (the fp16
round-to-int magic offset left in, UNMASKED — outlier columns carry the
rounded outlier activation, the same approximation the previous baseline
made) the output is
    y[m, o] = (sum_k qb[m, k] * Wc[k, o] + corr[o]) * xs[m] + bias[o]
where corr[o] = -1536 * sum_k Wc[k, o] cancels the magic offset. corr is
shipped as two fp16 rows (hi + lo residual, pre-scaled by 1/16 against
fp16 overflow) and applied by a k=2 matmul against a constant 16.0
column inside the same PSUM accumulation group.

Sharding: Wc/bias are sharded along out_features across the 8 cores
(column parallel); x and the ind-mask are replicated. Each core produces
its (512, 1376) output shard; the host concatenates.

Engine-queue separation (each queue is in-order, so phase-1 work of rep
r+1 must never sit behind phase-2 work of rep r):
    sync   : x loads + q transposes          (phase 1)
    vector : mask-mult/absmax/scales         (phase 1), y-out DMA issue
    scalar : quantize activations            (phase 1)
    gpsimd : weight loads + PSUM scale+bias  (phase 2)
    tensor : matmuls                         (phase 2)
Emission is software-pipelined (phase1(r+1) emitted before phase2(r)) so
rep r+1's quantization overlaps rep r's GEMM.
"""

import sys

import numpy as np

sys.path.insert(0, "/opt/trn_rl_repo")

import concourse.bass as bass  # noqa: E402
import concourse.mybir as mybir  # noqa: E402
import concourse.tile as tile  # noqa: E402
from concourse import bacc  # noqa: E402

N_CORES = 8
M = 512  # 8*64 rows
K = 4096  # in_features
OUT = 11008  # out_features
OSH = OUT // N_CORES  # 1376 per-core shard
FP = 256  # outlier columns
KT = K // 128  # 32 k-tiles
MT = M // 128  # 4 m-tiles
MAGIC = 1536.0  # fp16 spacing is 1.0 in [1024, 2048): forces round-to-int
CORR_S = 16.0  # corr rows are shipped as corr/CORR_S to stay in fp16 range
OC = 344  # o-chunk width (fits one PSUM bank: 344*4B <= 2KB)
NCH = OSH // OC  # 4 chunks
XH = 2048  # x streamed in half-rows
XQ = 1024  # absmax computed in chunks of this width

f32 = mybir.dt.float32
f16 = mybir.dt.float16
bf16 = mybir.dt.bfloat16
Alu = mybir.AluOpType
Act = mybir.ActivationFunctionType


def build_program(nrep=1, debug_dump=False):
    """Build the kernel program. nrep>1 emits the whole body nrep times
    (same inputs, same outputs) — used only to measure steady-state HW time
    as (t(nrep) - t(1)) / (nrep - 1)."""
    nc = bacc.Bacc(
        "TRN2", target_bir_lowering=False, debug=False, num_devices=N_CORES
    )

    x_d = nc.dram_tensor("x_in", [M, K], f32, kind="ExternalInput").ap()
    # host-packed combined weight: [chunk, partition(k%128), kk, o-in-chunk]
    w_d = nc.dram_tensor("w_in", [NCH, 128, KT * OC], f16, kind="ExternalInput").ap()
    mask_d = nc.dram_tensor("mask_in", [1, K], bf16, kind="ExternalInput").ap()
    # rows: corr_hi, corr_lo (magic-offset correction / CORR_S)
    corr_d = nc.dram_tensor("corr_in", [2, OSH], f16, kind="ExternalInput").ap()
    bias_d = nc.dram_tensor("bias_in", [1, OSH], f32, kind="ExternalInput").ap()
    y_d = nc.dram_tensor("y_out", [M, OSH], f32, kind="ExternalOutput").ap()
    dbg = {}
    if debug_dump:
        for nm, shape, dt in [
            ("dbg_scales", [128, 4 * MT], f32),
            ("dbg_q0", [128, KT * 128], f16),
            ("dbg_wt0", [128, KT * OC], f16),
        ]:
            dbg[nm] = nc.dram_tensor(nm, shape, dt, kind="ExternalOutput").ap()

    with tile.TileContext(nc) as tc:
        with (
            tc.tile_pool(name="persist", bufs=1) as persist,
            tc.tile_pool(name="xpool", bufs=2) as xpool,
            tc.tile_pool(name="xzpool", bufs=2) as xzpool,
            tc.tile_pool(name="qnpool", bufs=2) as qnpool,
            tc.tile_pool(name="wtpool", bufs=4) as wtpool,
            tc.tile_pool(name="ypool", bufs=4) as ypool,
            tc.tile_pool(name="psmain", bufs=4, space="PSUM") as psmain,
        ):
            # ---------- persistent tiles ----------
            # q^T (k-part, kk, m): one tile per m-tile, double-buffered
            # across reps so rep r+1's quantization overlaps rep r's GEMM.
            q_sets = []
            for par in range(2):
                qset = []
                for mt in range(MT):
                    q_t = persist.tile(
                        [128, KT, 128],
                        f16,
                        tag=f"qT{par}_{mt}",
                        name=f"qT{par}_{mt}",
                    )
                    qset.append(q_t)
                q_sets.append(qset)
            mask_bc = persist.tile([128, K], bf16)  # ind-mask broadcast
            corr_sb = persist.tile([2, OSH], f16)  # corr_hi / corr_lo
            ones2 = persist.tile([2, 128], f16)  # CORR_S-valued lhsT
            bias_bc = persist.tile([128, OSH], f32)  # bias broadcast
            am_parts = persist.tile([128, MT * (K // XQ)], f32)
            am_all = persist.tile([128, MT], f32)
            xs_all = persist.tile([128, 4 * MT], f32)  # rep%4-indexed
            recip_all = persist.tile([128, 4 * MT], f32)

            # ---------- setup ----------
            nc.gpsimd.dma_start(
                out=mask_bc,
                in_=bass.AP(mask_d.tensor, mask_d.offset, [[0, 128], [1, K]]),
            )
            nc.gpsimd.dma_start(out=corr_sb, in_=corr_d)
            nc.gpsimd.dma_start(
                out=bias_bc,
                in_=bass.AP(bias_d.tensor, bias_d.offset, [[0, 128], [1, OSH]]),
            )
            nc.gpsimd.memset(ones2, CORR_S)

            def phase1(rep):
                par = rep % 2
                pq = rep % 4
                q_tiles = q_sets[par]
                nhalf = K // XH  # 2
                nq = XH // XQ  # 2
                for mt in range(MT):
                    ms = slice(mt * 128, (mt + 1) * 128)
                    x_hs = []
                    for h in range(nhalf):
                        x_h = xpool.tile(
                            [128, XH], f32, tag="x", name=f"x_{rep}_{mt}_{h}"
                        )
                        nc.sync.dma_start(
                            out=x_h, in_=x_d[ms, h * XH : (h + 1) * XH]
                        )
                        x_hs.append(x_h)
                        for q in range(nq):
                            xz = xzpool.tile(
                                [128, XQ], f32, tag="xz", name=f"xz_{rep}_{mt}_{h}_{q}"
                            )
                            nc.vector.tensor_tensor(
                                out=xz,
                                in0=x_h[:, q * XQ : (q + 1) * XQ],
                                in1=mask_bc[
                                    :, (h * nq + q) * XQ : (h * nq + q + 1) * XQ
                                ],
                                op=Alu.mult,
                            )
                            pcol = mt * (K // XQ) + h * nq + q
                            nc.vector.tensor_reduce(
                                out=am_parts[:, pcol : pcol + 1],
                                in_=xz,
                                axis=mybir.AxisListType.X,
                                op=Alu.max,
                                apply_absolute_value=True,
                            )
                    nc.vector.tensor_reduce(
                        out=am_all[:, mt : mt + 1],
                        in_=am_parts[:, mt * (K // XQ) : (mt + 1) * (K // XQ)],
                        axis=mybir.AxisListType.X,
                        op=Alu.max,
                        apply_absolute_value=False,
                    )
                    pc = pq * MT + mt
                    # xs = max(absmax/127, 1e-8); recip = 1/xs
                    nc.vector.tensor_scalar(
                        xs_all[:, pc : pc + 1],
                        am_all[:, mt : mt + 1],
                        1.0 / 127.0,
                        1e-8,
                        Alu.mult,
                        Alu.max,
                    )
                    nc.vector.reciprocal(
                        out=recip_all[:, pc : pc + 1], in_=xs_all[:, pc : pc + 1]
                    )
                    q_t = q_tiles[mt]
                    for h in range(nhalf):
                        # qb = x*recip + 1536 -> fp16 write rounds to int (RNE)
                        qn = qnpool.tile(
                            [128, XH], f16, tag="qn", name=f"qn_{rep}_{mt}_{h}"
                        )
                        nc.scalar.activation(
                            out=qn,
                            in_=x_hs[h],
                            func=Act.Copy,
                            bias=MAGIC,
                            scale=recip_all[:, pc : pc + 1],
                        )
                        # transpose into q_t[:, k-half, :].
                        # NOTE: dma transpose must be issued from the SP
                        # sequencer — ACT-issued xbar transposes corrupt
                        # data on HW.
                        nc.sync.dma_start(
                            out=q_t[:, h * (XH // 128) : (h + 1) * (XH // 128), :],
                            in_=qn,
                            transpose=True,
                        )

            def load_w(rep, c):
                wt = wtpool.tile(
                    [128, KT, OC], f16, tag="wt", name=f"wt_{rep}_{c}"
                )
                nc.gpsimd.dma_start(out=wt, in_=w_d[c])
                return wt

            wt_next = {}

            def phase2(rep, prefetch_next):
                par = rep % 2
                pq = rep % 4
                q_tiles = q_sets[par]
                nonlocal wt_next
                wt_cur = wt_next if wt_next else {c: load_w(rep, c) for c in range(NCH)}
                wt_next = {}
                for c in range(NCH):
                    wt = wt_cur[c]
                    o0 = c * OC
                    for mt in range(MT):
                        ms = slice(mt * 128, (mt + 1) * 128)
                        pc = pq * MT + mt
                        ps = psmain.tile(
                            [128, OC], f32, tag="ps", name=f"ps_{rep}_{c}_{mt}"
                        )
                        for kk in range(KT):
                            nc.tensor.matmul(
                                ps,
                                lhsT=q_tiles[mt][:, kk, :],
                                rhs=wt[:, kk, :],
                                start=(kk == 0),
                                stop=False,
                            )
                        # cancel the +1536 magic offset:
                        # += [16, 16] . [corr_hi; corr_lo]
                        nc.tensor.matmul(
                            ps,
                            lhsT=ones2,
                            rhs=corr_sb[:, o0 : o0 + OC],
                            start=False,
                            stop=True,
                        )
                        ysb = ypool.tile(
                            [128, OC], f32, tag="ysb", name=f"ysb_{rep}_{c}_{mt}"
                        )
                        # y = ps * xs + bias
                        nc.vector.scalar_tensor_tensor(
                            out=ysb,
                            in0=ps,
                            scalar=xs_all[:, pc : pc + 1],
                            in1=bias_bc[:, o0 : o0 + OC],
                            op0=Alu.mult,
                            op1=Alu.add,
                        )
                        nc.scalar.dma_start(out=y_d[ms, o0 : o0 + OC], in_=ysb)
                    # prefetch next rep's chunk-c weights now that this
                    # rep's reads of the same wt buffer are emitted
                    if prefetch_next:
                        wt_next[c] = load_w(rep + 1, c)
                if debug_dump and rep == 0:
                    nc.sync.dma_start(out=dbg["dbg_scales"], in_=xs_all)
                    nc.sync.dma_start(out=dbg["dbg_q0"], in_=q_tiles[0][:, :, :])
                    nc.sync.dma_start(out=dbg["dbg_wt0"], in_=wt_cur[0][:, :, :])

            # software-pipelined emission: phase1(r+1) before phase2(r)
            phase1(0)
            for rep in range(1, nrep):
                phase1(rep)
                phase2(rep - 1, prefetch_next=(rep < nrep))
            phase2(nrep - 1, prefetch_next=False)

    nc.compile()
    return nc


_NC_CACHE = None


def get_program():
    global _NC_CACHE
    if _NC_CACHE is None:
        _NC_CACHE = build_program()
    return _NC_CACHE


def make_in_maps(x, q_weight, scale_col, weight_cache, ind, bias):
    x2 = np.ascontiguousarray(np.asarray(x, dtype=np.float32).reshape(M, K))
    q_weight = np.asarray(q_weight, dtype=np.int32)
    scale_col = np.asarray(scale_col, dtype=np.float32).reshape(OUT)
    weight_cache = np.asarray(weight_cache, dtype=np.float32)
    ind_np = np.asarray(ind, dtype=np.int32).reshape(FP)
    bias_np = np.asarray(bias, dtype=np.float32).reshape(OUT)

    import ml_dtypes

    mask = np.ones(K, dtype=np.float32)
    mask[ind_np] = 0.0
    mask_bf = mask.astype(ml_dtypes.bfloat16).reshape(1, K)

    # combined weight: WcT[k, o] = q_weight[o, k]*scale_col[o] off-outlier,
    # scatter-add of weight_cache on outlier rows (duplicates in ind add,
    # matching x[:, ind] gather + separate GEMM in the reference)
    wf = q_weight.astype(np.float32) * scale_col.reshape(OUT, 1)  # [OUT, K]
    wcT = np.ascontiguousarray(wf.T)  # [K, OUT]
    cr = np.zeros((K, OUT), dtype=np.float32)
    np.add.at(cr, ind_np, weight_cache.T.astype(np.float32))
    outlier_rows = np.zeros(K, dtype=bool)
    outlier_rows[ind_np] = True
    wcT[outlier_rows] = cr[outlier_rows]
    wc16 = wcT.astype(np.float16)  # [K, OUT]

    # magic-offset correction rows: corr = -MAGIC * colsum(Wc) / CORR_S,
    # split hi/lo so the k=2 fp16 matmul reproduces it almost exactly
    colsum = wc16.astype(np.float64).sum(axis=0)
    corr = -MAGIC * colsum / CORR_S
    corr_hi = corr.astype(np.float16)
    corr_lo = (corr - corr_hi.astype(np.float64)).astype(np.float16)
    assert np.abs(corr).max() < 60000.0, "corr overflow"

    in_maps = []
    for c in range(N_CORES):
        sl = slice(c * OSH, (c + 1) * OSH)
        shard = wc16[:, sl]  # [K, OSH]
        # pack: [chunk, partition(k%128), kk, o-in-chunk]
        wpack = np.ascontiguousarray(
            shard.reshape(KT, 128, NCH, OC).transpose(2, 1, 0, 3)
        ).reshape(NCH, 128, KT * OC)
        in_maps.append(
            {
                "x_in": x2,
                "w_in": wpack,
                "mask_in": mask_bf,
                "corr_in": np.ascontiguousarray(
                    np.stack([corr_hi[sl], corr_lo[sl]])
                ),
                "bias_in": np.ascontiguousarray(bias_np[sl].reshape(1, OSH)),
            }
        )
    return in_maps


def kernel(x, q_weight, scale_col, weight_cache, ind, bias):
    from concourse.bass_utils import run_bass_kernel_spmd

    nc = get_program()
    in_maps = make_in_maps(x, q_weight, scale_col, weight_cache, ind, bias)
    res = run_bass_kernel_spmd(nc, in_maps, core_ids=list(range(N_CORES)))
    shards = [res.results[c]["y_out"] for c in range(N_CORES)]
    y = np.concatenate(shards, axis=1)
    return y.reshape(8, 64, OUT).astype(np.float32)


# revision 16
# speedup vs baseline: 1.1318x; 1.1318x over previous
"""MixLinear GEMM kernel for Trainium2 (8 NeuronCores, column-parallel).

Computes, for full inputs:
    inputs = x.reshape(-1, 4096)
    act_outliers = inputs[:, ind]
    inputs_z = inputs with ind-columns zeroed
    x_scale = clamp(rowmax(|inputs_z|)/127, 1e-8)
    q_x = round(inputs_z / x_scale)                  (|q_x| <= 127 by construction)
    y = (q_x @ q_weight.T) * x_scale * scale_col + act_outliers @ weight_cache.T + bias

Device-side formulation: the host pre-packs a combined fp16 weight
    Wc[k, o] = q_weight[o, k] * scale_col[o]          for k not in ind
    Wc[k, o] = sum_{j: ind[j]==k} weight_cache[o, j]  for k in ind
so that with qb[m, k] = round(x[m, k] / xs[m]) + 1536 (the fp16
round-to-int magic offset left in, UNMASKED — outlier columns carry the
rounded outlier activation, the same approximation the previous baseline
made) the output is
    y[m, o] = (sum_k qb[m, k] * Wc[k, o] + corr[o]) * xs[m] + bias[o]
where corr[o] = -1536 * sum_k Wc[k, o] cancels the magic offset. corr is
shipped as two fp16 rows (hi + lo residual, pre-scaled by 1/16 against
fp16 overflow) and applied by a k=2 matmul against a constant 16.0
column inside the same PSUM accumulation group.

Sharding: Wc/bias are sharded along out_features across the 8 cores
(column parallel); x and the ind-mask are replicated. Each core produces
its (512, 1376) output shard; the host concatenates.

Engine-queue separation (each queue is in-order, so phase-1 work of rep
r+1 must never sit behind phase-2 work of rep r):
    sync   : x loads + q transposes          (phase 1)
    vector : mask-mult/absmax/scales         (phase 1), y-out DMA issue
    scalar : quantize activations            (phase 1)
    gpsimd : weight loads + PSUM scale+bias  (phase 2)
    tensor : matmuls                         (phase 2)
Emission is software-pipelined (phase1(r+1) emitted before phase2(r)) so
rep r+1's quantization overlaps rep r's GEMM.
"""

import sys

import numpy as np

sys.path.insert(0, "/opt/trn_rl_repo")

import concourse.bass as bass  # noqa: E402
import concourse.mybir as mybir  # noqa: E402
import concourse.tile as tile  # noqa: E402
from concourse import bacc  # noqa: E402

N_CORES = 8
M = 512  # 8*64 rows
K = 4096  # in_features
OUT = 11008  # out_features
OSH = OUT // N_CORES  # 1376 per-core shard
FP = 256  # outlier columns
KT = K // 128  # 32 k-tiles
MT = M // 128  # 4 m-tiles
MAGIC = 1536.0  # fp16 spacing is 1.0 in [1024, 2048): forces round-to-int
CORR_S = 16.0  # corr rows are shipped as corr/CORR_S to stay in fp16 range
OC = 344  # o-chunk width (fits one PSUM bank: 344*4B <= 2KB)
NCH = OSH // OC  # 4 chunks
XH = 2048  # x streamed in half-rows
XQ = 1024  # absmax computed in chunks of this width

f32 = mybir.dt.float32
f16 = mybir.dt.float16
bf16 = mybir.dt.bfloat16
Alu = mybir.AluOpType
Act = mybir.ActivationFunctionType


def build_program(nrep=1, debug_dump=False):
    """Build the kernel program. nrep>1 emits the whole body nrep times
    (same inputs, same outputs) — used only to measure steady-state HW time
    as (t(nrep) - t(1)) / (nrep - 1)."""
    nc = bacc.Bacc(
        "TRN2", target_bir_lowering=False, debug=False, num_devices=N_CORES
    )

    x_d = nc.dram_tensor("x_in", [M, K], f32, kind="ExternalInput").ap()
    # host-packed combined weight: [chunk, partition(k%128), kk, o-in-chunk]
    w_d = nc.dram_tensor("w_in", [NCH, 128, KT * OC], bf16, kind="ExternalInput").ap()
    mask_d = nc.dram_tensor("mask_in", [1, K], bf16, kind="ExternalInput").ap()
    bias_d = nc.dram_tensor("bias_in", [1, OSH], f32, kind="ExternalInput").ap()
    y_d = nc.dram_tensor("y_out", [M, OSH], f32, kind="ExternalOutput").ap()
    dbg = {}
    if debug_dump:
        for nm, shape, dt in [
            ("dbg_scales", [128, 4 * MT], f32),
            ("dbg_q0", [128, KT * 128], bf16),
            ("dbg_wt0", [128, KT * OC], bf16),
        ]:
            dbg[nm] = nc.dram_tensor(nm, shape, dt, kind="ExternalOutput").ap()

    with tile.TileContext(nc) as tc:
        with (
            tc.tile_pool(name="persist", bufs=1) as persist,
            tc.tile_pool(name="xpool", bufs=2) as xpool,
            tc.tile_pool(name="xzpool", bufs=2) as xzpool,
            tc.tile_pool(name="qnpool", bufs=2) as qnpool,
            tc.tile_pool(name="qbpool", bufs=2) as qbpool,
            tc.tile_pool(name="wtpool", bufs=4) as wtpool,
            tc.tile_pool(name="ypool", bufs=3) as ypool,
            tc.tile_pool(name="psmain", bufs=4, space="PSUM") as psmain,
        ):
            # ---------- persistent tiles ----------
            # q^T (k-part, kk, m): one tile per m-tile, double-buffered
            # across reps so rep r+1's quantization overlaps rep r's GEMM.
            q_sets = []
            for par in range(2):
                qset = []
                for mt in range(MT):
                    q_t = persist.tile(
                        [128, KT, 128],
                        bf16,
                        tag=f"qT{par}_{mt}",
                        name=f"qT{par}_{mt}",
                    )
                    qset.append(q_t)
                q_sets.append(qset)
            mask_bc = persist.tile([128, K], bf16)  # ind-mask broadcast
            bias_bc = persist.tile([128, OSH], f32)  # bias broadcast
            am_parts = persist.tile([128, MT * (K // XQ)], f32)
            am_all = persist.tile([128, MT], f32)
            xs_all = persist.tile([128, 4 * MT], f32)  # rep%4-indexed
            recip_all = persist.tile([128, 4 * MT], f32)

            # ---------- setup ----------
            nc.gpsimd.dma_start(
                out=mask_bc,
                in_=bass.AP(mask_d.tensor, mask_d.offset, [[0, 128], [1, K]]),
            )
            nc.gpsimd.dma_start(
                out=bias_bc,
                in_=bass.AP(bias_d.tensor, bias_d.offset, [[0, 128], [1, OSH]]),
            )

            def phase1(rep):
                par = rep % 2
                pq = rep % 4
                q_tiles = q_sets[par]
                nhalf = K // XH  # 2
                nq = XH // XQ  # 2
                for mt in range(MT):
                    ms = slice(mt * 128, (mt + 1) * 128)
                    x_hs = []
                    for h in range(nhalf):
                        x_h = xpool.tile(
                            [128, XH], f32, tag="x", name=f"x_{rep}_{mt}_{h}"
                        )
                        nc.sync.dma_start(
                            out=x_h, in_=x_d[ms, h * XH : (h + 1) * XH]
                        )
                        x_hs.append(x_h)
                        for q in range(nq):
                            xz = xzpool.tile(
                                [128, XQ], f32, tag="xz", name=f"xz_{rep}_{mt}_{h}_{q}"
                            )
                            nc.vector.tensor_tensor(
                                out=xz,
                                in0=x_h[:, q * XQ : (q + 1) * XQ],
                                in1=mask_bc[
                                    :, (h * nq + q) * XQ : (h * nq + q + 1) * XQ
                                ],
                                op=Alu.mult,
                            )
                            pcol = mt * (K // XQ) + h * nq + q
                            nc.vector.tensor_reduce(
                                out=am_parts[:, pcol : pcol + 1],
                                in_=xz,
                                axis=mybir.AxisListType.X,
                                op=Alu.max,
                                apply_absolute_value=True,
                            )
                    nc.vector.tensor_reduce(
                        out=am_all[:, mt : mt + 1],
                        in_=am_parts[:, mt * (K // XQ) : (mt + 1) * (K // XQ)],
                        axis=mybir.AxisListType.X,
                        op=Alu.max,
                        apply_absolute_value=False,
                    )
                    pc = pq * MT + mt
                    # xs = max(absmax/127, 1e-8); recip = 1/xs
                    nc.vector.tensor_scalar(
                        xs_all[:, pc : pc + 1],
                        am_all[:, mt : mt + 1],
                        1.0 / 127.0,
                        1e-8,
                        Alu.mult,
                        Alu.max,
                    )
                    nc.vector.reciprocal(
                        out=recip_all[:, pc : pc + 1], in_=xs_all[:, pc : pc + 1]
                    )
                    q_t = q_tiles[mt]
                    for h in range(nhalf):
                        # pass 1: x*recip + 1536 -> fp16 write rounds to
                        # int (RNE); pass 2: subtract the magic, store the
                        # exact small ints as bf16 for the GEMM
                        qn = qnpool.tile(
                            [128, XH], f16, tag="qn", name=f"qn_{rep}_{mt}_{h}"
                        )
                        nc.scalar.activation(
                            out=qn,
                            in_=x_hs[h],
                            func=Act.Copy,
                            bias=MAGIC,
                            scale=recip_all[:, pc : pc + 1],
                        )
                        qb = qbpool.tile(
                            [128, XH], bf16, tag="qb", name=f"qb_{rep}_{mt}_{h}"
                        )
                        nc.scalar.activation(
                            out=qb,
                            in_=qn,
                            func=Act.Copy,
                            bias=-MAGIC,
                            scale=1.0,
                        )
                        # transpose into q_t[:, k-half, :].
                        # NOTE: dma transpose must be issued from the SP
                        # sequencer — ACT-issued xbar transposes corrupt
                        # data on HW.
                        nc.sync.dma_start(
                            out=q_t[:, h * (XH // 128) : (h + 1) * (XH // 128), :],
                            in_=qb,
                            transpose=True,
                        )

            def load_w(rep, c):
                wt = wtpool.tile(
                    [128, KT, OC], bf16, tag="wt", name=f"wt_{rep}_{c}"
                )
                nc.gpsimd.dma_start(out=wt, in_=w_d[c])
                return wt

            wt_next = {}

            def phase2(rep, prefetch_next):
                par = rep % 2
                pq = rep % 4
                q_tiles = q_sets[par]
                nonlocal wt_next
                wt_cur = wt_next if wt_next else {c: load_w(rep, c) for c in range(NCH)}
                wt_next = {}
                for c in range(NCH):
                    wt = wt_cur[c]
                    o0 = c * OC
                    for mt in range(MT):
                        ms = slice(mt * 128, (mt + 1) * 128)
                        pc = pq * MT + mt
                        ps = psmain.tile(
                            [128, OC], f32, tag="ps", name=f"ps_{rep}_{c}_{mt}"
                        )
                        for kk in range(KT):
                            nc.tensor.matmul(
                                ps,
                                lhsT=q_tiles[mt][:, kk, :],
                                rhs=wt[:, kk, :],
                                start=(kk == 0),
                                stop=(kk == KT - 1),
                            )
                        ysb = ypool.tile(
                            [128, OC], f32, tag="ysb", name=f"ysb_{rep}_{c}_{mt}"
                        )
                        # y = ps * xs + bias
                        nc.vector.scalar_tensor_tensor(
                            out=ysb,
                            in0=ps,
                            scalar=xs_all[:, pc : pc + 1],
                            in1=bias_bc[:, o0 : o0 + OC],
                            op0=Alu.mult,
                            op1=Alu.add,
                        )
                        nc.scalar.dma_start(out=y_d[ms, o0 : o0 + OC], in_=ysb)
                    # prefetch next rep's chunk-c weights now that this
                    # rep's reads of the same wt buffer are emitted
                    if prefetch_next:
                        wt_next[c] = load_w(rep + 1, c)
                if debug_dump and rep == 0:
                    nc.sync.dma_start(out=dbg["dbg_scales"], in_=xs_all)
                    nc.sync.dma_start(out=dbg["dbg_q0"], in_=q_tiles[0][:, :, :])
                    nc.sync.dma_start(out=dbg["dbg_wt0"], in_=wt_cur[0][:, :, :])

            # software-pipelined emission: phase1(r+1) before phase2(r)
            phase1(0)
            for rep in range(1, nrep):
                phase1(rep)
                phase2(rep - 1, prefetch_next=(rep < nrep))
            phase2(nrep - 1, prefetch_next=False)

    nc.compile()
    return nc


_NC_CACHE = None


def get_program():
    global _NC_CACHE
    if _NC_CACHE is None:
        _NC_CACHE = build_program()
    return _NC_CACHE


def make_in_maps(x, q_weight, scale_col, weight_cache, ind, bias):
    x2 = np.ascontiguousarray(np.asarray(x, dtype=np.float32).reshape(M, K))
    q_weight = np.asarray(q_weight, dtype=np.int32)
    scale_col = np.asarray(scale_col, dtype=np.float32).reshape(OUT)
    weight_cache = np.asarray(weight_cache, dtype=np.float32)
    ind_np = np.asarray(ind, dtype=np.int32).reshape(FP)
    bias_np = np.asarray(bias, dtype=np.float32).reshape(OUT)

    import ml_dtypes

    mask = np.ones(K, dtype=np.float32)
    mask[ind_np] = 0.0
    mask_bf = mask.astype(ml_dtypes.bfloat16).reshape(1, K)

    # combined weight: WcT[k, o] = q_weight[o, k]*scale_col[o] off-outlier,
    # scatter-add of weight_cache on outlier rows (duplicates in ind add,
    # matching x[:, ind] gather + separate GEMM in the reference)
    wf = q_weight.astype(np.float32) * scale_col.reshape(OUT, 1)  # [OUT, K]
    wcT = np.ascontiguousarray(wf.T)  # [K, OUT]
    cr = np.zeros((K, OUT), dtype=np.float32)
    np.add.at(cr, ind_np, weight_cache.T.astype(np.float32))
    outlier_rows = np.zeros(K, dtype=bool)
    outlier_rows[ind_np] = True
    wcT[outlier_rows] = cr[outlier_rows]
    wc16 = wcT.astype(ml_dtypes.bfloat16)  # [K, OUT]

    in_maps = []
    for c in range(N_CORES):
        sl = slice(c * OSH, (c + 1) * OSH)
        shard = wc16[:, sl]  # [K, OSH]
        # pack: [chunk, partition(k%128), kk, o-in-chunk]
        wpack = np.ascontiguousarray(
            shard.reshape(KT, 128, NCH, OC).transpose(2, 1, 0, 3)
        ).reshape(NCH, 128, KT * OC)
        in_maps.append(
            {
                "x_in": x2,
                "w_in": wpack,
                "mask_in": mask_bf,
                "bias_in": np.ascontiguousarray(bias_np[sl].reshape(1, OSH)),
            }
        )
    return in_maps


def kernel(x, q_weight, scale_col, weight_cache, ind, bias):
    from concourse.bass_utils import run_bass_kernel_spmd

    nc = get_program()
    in_maps = make_in_maps(x, q_weight, scale_col, weight_cache, ind, bias)
    res = run_bass_kernel_spmd(nc, in_maps, core_ids=list(range(N_CORES)))
    shards = [res.results[c]["y_out"] for c in range(N_CORES)]
    y = np.concatenate(shards, axis=1)
    return y.reshape(8, 64, OUT).astype(np.float32)
